# revision 15
# baseline (speedup 1.0000x reference)
"""Trainium2 Bass kernel: per-cluster PCA geometry features (segment reduce).

Problem: data [4194304, 6] f32, clusts [32768, 128] int — per cluster of 128
voxels compute: center (mean of xyz), normalized covariance B = A/lmax,
principal axis v0 scaled by dirwt = 1 - lmid/lmax with a sign fix, size.

Strategy (v4): shard the 32768 clusters across 8 NeuronCores (4096 each).
Host pre-gathers each cluster's voxel coords (pure permutation), casts to
bf16, and ships TWO layouts per core:
  voxel-major  xt/yt/zt [128 vox, 4096 clusters] — phase-1 moment sums run
    on the PE (column sums via ones-rhs matmuls, nearly free).
  cluster-major xc/yc/zc [128 part, 128 vox, 16 seg] per half — phase-2
    element ops. Segment-INNERMOST layout keeps every DVE operand's last AP
    dim stride-1 so bf16 ops hit the 2x DVE mode, including per-cluster
    broadcasts (stride-0 on the middle/voxel dim only).
Cluster c = g*128 + q maps to (partition q, segment g), matching the PE
column-sum output layout, so moments land directly where the eigensolve
([128, 32] fp32 small-tile analytic 3x3 solve, trig method) wants them.
Input DMAs are split across the SP/ACT/Pool issue queues so transfers
overlap; work is split across DVE/ACT/Pool by measured cost-model rates
(Pool subtract is cheaper than mult); ACT table switches (sqrt<->trig
sets) are batched; feature values are written straight into the output
tile; tails and output DMA run per half.
"""
import numpy as np
from contextlib import ExitStack

import concourse.bass as bass
import concourse.bacc as bacc
import concourse.tile as tile
from concourse import mybir
from concourse.bass_utils import run_bass_kernel_spmd

N_CLUSTS = 32768
CLUST_SIZE = 128
N_CORES = 8
C_LOC = N_CLUSTS // N_CORES   # 4096 clusters per core
P = 128                       # SBUF partitions
NSEG = C_LOC // P             # 32 clusters (segments) per partition
V = CLUST_SIZE                # 128 voxels per cluster
NH = 2                        # halves for pipelining
GH = NSEG // NH               # 16 segments per half
CH = C_LOC // NH              # 2048 clusters per half

F32 = mybir.dt.float32
BF16 = mybir.dt.bfloat16
U8 = mybir.dt.uint8
AF = mybir.ActivationFunctionType
OP = mybir.AluOpType
AX = mybir.AxisListType

PI_2 = 1.5707963267948966
PI_6 = 0.5235987755982988

_CACHED = {}


def build_nc():
    nc = bacc.Bacc()
    xt_d = nc.dram_tensor("xt", [V, C_LOC], BF16, kind="ExternalInput").ap()
    yt_d = nc.dram_tensor("yt", [V, C_LOC], BF16, kind="ExternalInput").ap()
    zt_d = nc.dram_tensor("zt", [V, C_LOC], BF16, kind="ExternalInput").ap()
    xc_d = nc.dram_tensor("xc", [NH, P, V, GH], BF16, kind="ExternalInput").ap()
    yc_d = nc.dram_tensor("yc", [NH, P, V, GH], BF16, kind="ExternalInput").ap()
    zc_d = nc.dram_tensor("zc", [NH, P, V, GH], BF16, kind="ExternalInput").ap()
    feats_d = nc.dram_tensor("feats", [NSEG, P, 16], F32, kind="ExternalOutput").ap()

    with tile.TileContext(nc) as tc, ExitStack() as ctx:
        pool = ctx.enter_context(tc.tile_pool(name="main", bufs=1))
        sp = ctx.enter_context(tc.tile_pool(name="p1s", bufs=6))
        p2p = ctx.enter_context(tc.tile_pool(name="p2s", bufs=1))
        pp = ctx.enter_context(tc.tile_pool(name="psum", bufs=2, space="PSUM"))

        D = nc.vector   # DVE
        A = nc.scalar   # Activation
        G = nc.gpsimd   # Pool

        ones = pool.tile([P, 1], BF16, tag="ones")
        G.memset(ones[:], 1.0)
        bias_pi2 = pool.tile([P, 1], F32, tag="bias_pi2")
        bias_pi6 = pool.tile([P, 1], F32, tag="bias_pi6")
        G.memset(bias_pi2[:], PI_2)
        G.memset(bias_pi6[:], PI_6)
        A.activation(bias_pi6[:], bias_pi2[:], AF.Sqrt)
        G.memset(bias_pi6[:], PI_6)

        # ---- input DMAs, split across issue queues so transfers overlap ----
        vm = {}   # (coord, half) -> [P, CH] bf16 voxel-major
        cm = {}   # (coord, half) -> [P, V, GH] bf16 cluster-major seg-inner
        for h in range(NH):
            veng = nc.sync if h == 0 else nc.gpsimd
            for k, (name, d) in enumerate(
                    (("x", xt_d), ("y", yt_d), ("z", zt_d))):
                t = pool.tile([P, CH], BF16, tag=f"vm_{name}{h}", name=f"vm_{name}{h}")
                veng.dma_start(t[:], d[:, h * CH:(h + 1) * CH])
                vm[(k, h)] = t
        for h in range(NH):
            eng = nc.scalar if h == 0 else nc.sync
            for k, (name, d) in enumerate(
                    (("x", xc_d), ("y", yc_d), ("z", zc_d))):
                t = pool.tile([P, V, GH], BF16, tag=f"cm_{name}{h}", name=f"cm_{name}{h}")
                eng.dma_start(t[:], d[h])
                cm[(k, h)] = t

        # ---- shared tiles / helpers ----
        ps = [pp.tile([P, 9 * GH], F32, tag=f"ps{h}", name=f"ps{h}")
              for h in range(NH)]
        moments = pool.tile([P, 9, NSEG], F32, tag="moments")
        Sx = moments[:, 0]; Sy = moments[:, 1]; Sz = moments[:, 2]
        Mxx = moments[:, 3]; Myy = moments[:, 4]; Mzz = moments[:, 5]
        Mxy = moments[:, 6]; Mxz = moments[:, 7]; Myz = moments[:, 8]

        feats = pool.tile([P, NSEG, 16], F32, tag="feats")

        def small(name, dt=F32):
            return pool.tile([P, NSEG], dt, tag=f"s_{name}", name=name)

        def ap(x):
            return x[:] if hasattr(x, "tag") else x

        def tt(eng, out, a, b, op):
            eng.tensor_tensor(ap(out), ap(a), ap(b), op)

        def ts(eng, out, in0, s1, s2=None, op0=OP.mult, op1=None):
            kw = dict(out=ap(out), in0=ap(in0), scalar1=s1, scalar2=s2, op0=op0)
            if op1 is not None:
                kw["op1"] = op1
            eng.tensor_scalar(**kw)

        def stt(eng, out, in0, s, in1, op0, op1):
            eng.scalar_tensor_tensor(out=ap(out), in0=ap(in0), scalar=s,
                                     in1=ap(in1), op0=op0, op1=op1)

        inv_s = 1.0 / V
        cxb = small("cxb", BF16); cyb = small("cyb", BF16); czb = small("czb", BF16)

        # ---- phase 1: moments via ACT/DVE/Pool products + PE column sums ----
        def colsum(h, plane, k):
            # column sums of [128, CH] plane: group g -> psum[:, k*GH+g]
            for g in range(GH):
                nc.tensor.matmul(
                    out=ps[h][:, k * GH + g: k * GH + g + 1],
                    lhsT=plane[:, g * P:(g + 1) * P],
                    rhs=ones[:, 0:1], start=True, stop=True)

        def p1_steps(h):
            x, y, z = vm[(0, h)], vm[(1, h)], vm[(2, h)]
            sqx = sp.tile([P, CH], BF16, tag="p1s", name=f"sqx{h}")
            sqy = sp.tile([P, CH], BF16, tag="p1s", name=f"sqy{h}")
            sqz = sp.tile([P, CH], BF16, tag="p1s", name=f"sqz{h}")
            cxy = sp.tile([P, CH], BF16, tag="p1s", name=f"cxy{h}")
            cxz = sp.tile([P, CH], BF16, tag="p1s", name=f"cxz{h}")
            cyz = sp.tile([P, CH], BF16, tag="p1s", name=f"cyz{h}")
            def st1():
                colsum(h, x, 0)
                D.tensor_tensor(sqx[:], x[:], x[:], OP.mult)
                colsum(h, sqx, 3)
            yield st1
            def st2():
                colsum(h, y, 1)
                A.activation(sqy[:], y[:], AF.Square)
                colsum(h, sqy, 4)
                D.tensor_tensor(cxy[:], x[:], y[:], OP.mult)
                colsum(h, cxy, 6)
            yield st2
            def st3():
                colsum(h, z, 2)
                G.tensor_tensor(sqz[:], z[:], z[:], OP.mult)
                colsum(h, sqz, 5)
                G.tensor_tensor(cxz[:], x[:], z[:], OP.mult)
                colsum(h, cxz, 7)
                G.tensor_tensor(cyz[:], y[:], z[:], OP.mult)
                colsum(h, cyz, 8)
            yield st3
            def st4():
                D.tensor_copy(
                    moments[:, :, h * GH:(h + 1) * GH],
                    ps[h][:].rearrange("p (k g) -> p k g", k=9))
                hs = slice(h * GH, (h + 1) * GH)
                ts(D, feats[:, hs, 0], Sx[:, hs], inv_s)
                ts(D, feats[:, hs, 1], Sy[:, hs], inv_s)
                ts(D, feats[:, hs, 2], Sz[:, hs], inv_s)
                D.tensor_copy(cxb[:, hs], feats[:, hs, 0])
                D.tensor_copy(cyb[:, hs], feats[:, hs, 1])
                D.tensor_copy(czb[:, hs], feats[:, hs, 2])
            yield st4

        def zipper(gens):
            done = [False] * len(gens)
            while not all(done):
                for i, g in enumerate(gens):
                    if done[i]:
                        continue
                    try:
                        next(g)()
                    except StopIteration:
                        done[i] = True

        zipper([p1_steps(0), p1_steps(1)])

        # ---- phase 2a in 4 zippered quarters (needs only the centers) ----
        NQ = 4
        GQ = NSEG // NQ   # 8 segments per quarter

        def bcq(t, qq):
            return t[:, None, qq * GQ:(qq + 1) * GQ].broadcast_to([P, V, GQ])

        Xc = [None] * NQ; Yc = [None] * NQ; Zc = [None] * NQ; Ssum = [None] * NQ

        def cmq(k, qq):
            h, r = divmod(qq, NQ // NH)
            return cm[(k, h)][:, :, r * GQ:(r + 1) * GQ]

        def p2a_steps(qq):
            Xc[qq] = p2p.tile([P, V, GQ], BF16, tag=f"Xc{qq}", name=f"Xc{qq}")
            Yc[qq] = p2p.tile([P, V, GQ], BF16, tag=f"Yc{qq}", name=f"Yc{qq}")
            Zc[qq] = p2p.tile([P, V, GQ], BF16, tag=f"Zc{qq}", name=f"Zc{qq}")
            yield lambda: G.tensor_tensor(Xc[qq][:], cmq(0, qq), bcq(cxb, qq), OP.subtract)
            yield lambda: G.tensor_tensor(Yc[qq][:], cmq(1, qq), bcq(cyb, qq), OP.subtract)
            yield lambda: G.tensor_tensor(Zc[qq][:], cmq(2, qq), bcq(czb, qq), OP.subtract)
            sx = p2p.tile([P, V, GQ], BF16, tag=f"sx{qq}", name=f"sx{qq}")
            sy = p2p.tile([P, V, GQ], BF16, tag=f"sy{qq}", name=f"sy{qq}")
            sz = p2p.tile([P, V, GQ], BF16, tag=f"sz{qq}", name=f"sz{qq}")
            yield lambda: A.activation(sx[:], Xc[qq][:], AF.Square)
            yield lambda: A.activation(sy[:], Yc[qq][:], AF.Square)
            yield lambda: D.tensor_tensor(sz[:], Zc[qq][:], Zc[qq][:], OP.mult)
            yield lambda: D.tensor_tensor(sx[:], sx[:], sy[:], OP.add)
            Ssum[qq] = p2p.tile([P, V, GQ], BF16, tag=f"s{qq}", name=f"s{qq}")
            yield lambda: G.tensor_tensor(Ssum[qq][:], sx[:], sz[:], OP.add)

        zipper([p2a_steps(qq) for qq in range(NQ)])

        # ---- eigen stage E-rest: [128, 32] fp32 analytic 3x3 eigensolve ----
        axx = small("axx"); ayy = small("ayy"); azz = small("azz")
        axy = small("axy"); axz = small("axz"); ayz = small("ayz")
        t0 = small("t0"); t1 = small("t1"); t2 = small("t2")
        t3 = small("t3"); t4 = small("t4"); t5 = small("t5")
        tt(D, t0, Sx, Sx, OP.mult)
        stt(D, axx, t0, -inv_s, Mxx, OP.mult, OP.add)
        tt(D, t1, Sy, Sy, OP.mult)
        stt(D, ayy, t1, -inv_s, Myy, OP.mult, OP.add)
        tt(D, t2, Sz, Sz, OP.mult)
        stt(D, azz, t2, -inv_s, Mzz, OP.mult, OP.add)
        tt(D, t3, Sx, Sy, OP.mult)
        stt(D, axy, t3, -inv_s, Mxy, OP.mult, OP.add)
        tt(D, t4, Sx, Sz, OP.mult)
        stt(D, axz, t4, -inv_s, Mxz, OP.mult, OP.add)
        tt(D, t5, Sy, Sz, OP.mult)
        stt(D, ayz, t5, -inv_s, Myz, OP.mult, OP.add)

        q = small("q")
        tt(D, t0, axx, ayy, OP.add)
        tt(D, t0, t0, azz, OP.add)
        ts(D, q, t0, 1.0 / 3.0)
        b11 = small("b11"); b22 = small("b22"); b33 = small("b33")
        tt(D, b11, axx, q, OP.subtract)
        tt(D, b22, ayy, q, OP.subtract)
        tt(D, b33, azz, q, OP.subtract)

        tt(G, t0, b11, b11, OP.mult)
        tt(G, t1, b22, b22, OP.mult)
        tt(G, t2, b33, b33, OP.mult)
        tt(G, t3, axy, axy, OP.mult)
        tt(G, t4, axz, axz, OP.mult)
        tt(G, t5, ayz, ayz, OP.mult)
        tt(D, t0, t0, t1, OP.add)
        tt(D, t0, t0, t2, OP.add)
        tt(G, t3, t3, t4, OP.add)
        tt(G, t3, t3, t5, OP.add)
        p2t = small("p2t")
        stt(D, p2t, t3, 2.0, t0, OP.mult, OP.add)
        p_ = small("p_")
        A.activation(p_[:], p2t[:], AF.Sqrt, scale=1.0 / 6.0)
        invp = small("invp")
        D.reciprocal(invp[:], p_[:])

        c11 = small("c11"); c22 = small("c22"); c33 = small("c33")
        c12 = small("c12"); c13 = small("c13"); c23 = small("c23")
        tt(D, c11, b11, invp, OP.mult)
        tt(D, c22, b22, invp, OP.mult)
        tt(D, c33, b33, invp, OP.mult)
        tt(G, c12, axy, invp, OP.mult)
        tt(G, c13, axz, invp, OP.mult)
        tt(G, c23, ayz, invp, OP.mult)

        # det(C)/2 -> r, clamped to [-1, 1]
        tt(D, t0, c22, c33, OP.mult)
        tt(G, t1, c23, c23, OP.mult)
        tt(D, t0, t0, t1, OP.subtract)
        tt(D, t0, t0, c11, OP.mult)
        tt(G, t2, c12, c33, OP.mult)
        tt(G, t3, c23, c13, OP.mult)
        tt(G, t2, t2, t3, OP.subtract)
        tt(G, t2, t2, c12, OP.mult)
        tt(D, t4, c12, c23, OP.mult)
        tt(D, t5, c22, c13, OP.mult)
        tt(D, t4, t4, t5, OP.subtract)
        tt(D, t4, t4, c13, OP.mult)
        tt(D, t0, t0, t2, OP.subtract)
        tt(D, t0, t0, t4, OP.add)
        r = small("r")
        ts(D, r, t0, 0.5, 1.0, OP.mult, OP.min)
        ts(D, r, r, -1.0, None, OP.max)

        # theta/4 = arctan(sqrt((1-r)/2) / (1 + sqrt((1+r)/2)))
        ts(D, t0, r, -0.5, 0.5, OP.mult, OP.add)
        ts(D, t1, r, 0.5, 0.5, OP.mult, OP.add)
        sa = small("sa"); sb = small("sb")
        A.activation(sa[:], t0[:], AF.Sqrt)
        A.activation(sb[:], t1[:], AF.Sqrt)
        ts(D, sb, sb, 1.0, None, OP.add)
        D.reciprocal(t2[:], sb[:])
        tt(D, t3, sa, t2, OP.mult)
        at4 = small("at4")
        A.activation(at4[:], t3[:], AF.Arctan)
        cmax = small("cmax"); smin = small("smin")
        A.activation(cmax[:], at4[:], AF.Sin, bias=bias_pi2[:, 0:1], scale=-4.0 / 3.0)
        A.activation(smin[:], at4[:], AF.Sin, bias=bias_pi6[:, 0:1], scale=4.0 / 3.0)

        w3 = small("w3"); w2 = small("w2")
        tt(D, t0, p_, cmax, OP.mult)
        stt(D, w3, t0, 2.0, q, OP.mult, OP.add)
        tt(G, t1, p_, smin, OP.mult)
        stt(D, t1, t1, -2.0, q, OP.mult, OP.add)      # w1
        stt(D, t2, q, 3.0, w3, OP.mult, OP.subtract)  # 3q - w3
        tt(D, w2, t2, t1, OP.subtract)
        invw3 = small("invw3")
        D.reciprocal(invw3[:], w3[:])
        dirwt = small("dirwt")
        tt(D, t0, w2, invw3, OP.mult)
        ts(D, dirwt, t0, -1.0, 1.0, OP.mult, OP.add)

        # B = A / w3 (9 entries, symmetric) written straight into feats
        tt(D, feats[:, :, 3], axx, invw3, OP.mult)
        tt(D, feats[:, :, 4], axy, invw3, OP.mult)
        A.copy(feats[:, :, 6], feats[:, :, 4])
        tt(D, feats[:, :, 5], axz, invw3, OP.mult)
        A.copy(feats[:, :, 9], feats[:, :, 5])
        tt(D, feats[:, :, 7], ayy, invw3, OP.mult)
        tt(D, feats[:, :, 8], ayz, invw3, OP.mult)
        A.copy(feats[:, :, 10], feats[:, :, 8])
        tt(D, feats[:, :, 11], azz, invw3, OP.mult)

        # principal eigenvector: cross products of rows of (A - w3 I)
        d1 = small("d1"); d2 = small("d2"); d3 = small("d3")
        tt(D, d1, axx, w3, OP.subtract)
        tt(D, d2, ayy, w3, OP.subtract)
        tt(D, d3, azz, w3, OP.subtract)
        u1 = small("u1"); u2 = small("u2"); u3 = small("u3")
        tt(G, t0, axy, ayz, OP.mult)
        tt(G, t1, d2, axz, OP.mult)
        tt(G, u1, t0, t1, OP.subtract)
        tt(D, t2, axy, axz, OP.mult)
        tt(D, t3, d1, ayz, OP.mult)
        tt(D, u2, t2, t3, OP.subtract)
        tt(G, t4, d1, d2, OP.mult)
        tt(G, t5, axy, axy, OP.mult)
        tt(G, u3, t4, t5, OP.subtract)
        k1 = small("k1"); k2 = small("k2")
        tt(D, t0, d2, d3, OP.mult)
        tt(D, t1, ayz, ayz, OP.mult)
        tt(D, k1, t0, t1, OP.subtract)
        tt(G, t2, ayz, axz, OP.mult)
        tt(G, t3, axy, d3, OP.mult)
        tt(G, k2, t2, t3, OP.subtract)
        # k3 = u1 (same formula)
        nu = small("nu"); nk = small("nk")
        nu1 = small("nu1")
        tt(D, nu1, u1, u1, OP.mult)
        tt(D, t0, u2, u2, OP.mult)
        tt(D, t1, u3, u3, OP.mult)
        tt(D, t0, t0, t1, OP.add)
        tt(D, nu, t0, nu1, OP.add)
        tt(G, t2, k1, k1, OP.mult)
        tt(G, t3, k2, k2, OP.mult)
        tt(G, t2, t2, t3, OP.add)
        tt(G, nk, t2, nu1, OP.add)
        m = small("m", U8)
        tt(D, m, nk, nu, OP.is_gt)
        e1 = small("e1"); e2 = small("e2"); e3 = small("e3"); ne = small("ne")
        D.select(e1[:], m[:], k1[:], u1[:])
        D.select(e2[:], m[:], k2[:], u2[:])
        D.select(e3[:], m[:], u1[:], u3[:])
        D.select(ne[:], m[:], nk[:], nu[:])
        rsn = small("rsn")
        A.activation(rsn[:], ne[:], AF.Sqrt)
        ts(D, rsn, rsn, 1e-30, None, OP.max)
        invn = small("invn")
        D.reciprocal(invn[:], rsn[:])
        v0x = small("v0x"); v0y = small("v0y"); v0z = small("v0z")
        tt(D, v0x, e1, invn, OP.mult)
        tt(D, v0y, e2, invn, OP.mult)
        tt(D, v0z, e3, invn, OP.mult)
        v0xb = small("v0xb", BF16); v0yb = small("v0yb", BF16)
        v0zb = small("v0zb", BF16)
        D.tensor_copy(v0xb[:], v0x[:])
        D.tensor_copy(v0yb[:], v0y[:])
        D.tensor_copy(v0zb[:], v0z[:])

        # ---- phase 2b: projections, residual norms, sign criterion ----
        sc = small("sc")
        G.memset(feats[:, :, 15], float(V))

        def p2b_steps(qq):
            a1 = p2p.tile([P, V, GQ], BF16, tag=f"a1{qq}", name=f"a1{qq}")
            a2 = p2p.tile([P, V, GQ], BF16, tag=f"a2{qq}", name=f"a2{qq}")
            a3 = p2p.tile([P, V, GQ], BF16, tag=f"a3{qq}", name=f"a3{qq}")
            yield lambda: D.tensor_tensor(a1[:], Xc[qq][:], bcq(v0xb, qq), OP.mult)
            yield lambda: G.tensor_tensor(a2[:], Yc[qq][:], bcq(v0yb, qq), OP.mult)
            yield lambda: G.tensor_tensor(a3[:], Zc[qq][:], bcq(v0zb, qq), OP.mult)
            x0 = p2p.tile([P, V, GQ], BF16, tag=f"x0{qq}", name=f"x0{qq}")
            yield lambda: D.tensor_tensor(x0[:], a1[:], a2[:], OP.add)
            yield lambda: D.tensor_tensor(x0[:], x0[:], a3[:], OP.add)
            q2 = p2p.tile([P, V, GQ], BF16, tag=f"q2{qq}", name=f"q2{qq}")
            yield lambda: A.activation(q2[:], x0[:], AF.Square)
            yield lambda: G.tensor_tensor(q2[:], Ssum[qq][:], q2[:], OP.subtract)
            yield lambda: ts(D, q2, q2, 0.0, None, OP.max)
            yield lambda: A.activation(q2[:], q2[:], AF.Sqrt)
            yield lambda: D.tensor_tensor(x0[:], x0[:], q2[:], OP.mult)
            yield lambda: D.tensor_reduce(sc[:, qq * GQ:(qq + 1) * GQ],
                                          x0[:].rearrange("p v g -> p g v"),
                                          axis=AX.X, op=OP.add)
            qs = slice(qq * GQ, (qq + 1) * GQ)
            yield lambda: ts(D, t0[:, qs], sc[:, qs], 0.0, -2.0, OP.is_lt, OP.mult)
            yield lambda: ts(D, t0[:, qs], t0[:, qs], 1.0, None, OP.add)
            yield lambda: tt(D, t1[:, qs], t0[:, qs], dirwt[:, qs], OP.mult)
            yield lambda: tt(D, feats[:, qs, 12], v0x[:, qs], t1[:, qs], OP.mult)
            yield lambda: tt(D, feats[:, qs, 13], v0y[:, qs], t1[:, qs], OP.mult)
            yield lambda: tt(D, feats[:, qs, 14], v0z[:, qs], t1[:, qs], OP.mult)
            yield lambda: nc.sync.dma_start(
                feats_d[qq * GQ:(qq + 1) * GQ].rearrange("g q f -> q g f"),
                feats[:, qs, :])

        zipper([p2b_steps(qq) for qq in range(NQ)])

    if not nc.is_finalized():
        nc.finalize()
    return nc


def kernel(data: np.ndarray, clusts: np.ndarray) -> np.ndarray:
    import ml_dtypes
    data = np.asarray(data, dtype=np.float32)
    clusts_np = np.asarray(clusts)
    C, S = clusts_np.shape
    assert (C, S) == (N_CLUSTS, CLUST_SIZE), (C, S)

    vox = data[:, 1:4]
    g3 = vox[clusts_np.reshape(-1).astype(np.int64)].reshape(C, S, 3)
    g3 = g3.astype(ml_dtypes.bfloat16)

    if "nc" not in _CACHED:
        _CACHED["nc"] = build_nc()
    nc = _CACHED["nc"]

    in_maps = []
    for c in range(N_CORES):
        a = g3[c * C_LOC:(c + 1) * C_LOC]          # [4096, 128, 3]
        vmt = np.ascontiguousarray(a.transpose(1, 0, 2))  # [128 vox, 4096, 3]
        # cluster-major seg-inner: [h, q, v, g] with c = (h*GH+g)*128 + q
        b = a.reshape(NH, GH, P, V, 3).transpose(0, 2, 3, 1, 4)
        b = np.ascontiguousarray(b)                # [2, 128, 128, 16, 3]
        in_maps.append({
            "xt": np.ascontiguousarray(vmt[:, :, 0]),
            "yt": np.ascontiguousarray(vmt[:, :, 1]),
            "zt": np.ascontiguousarray(vmt[:, :, 2]),
            "xc": np.ascontiguousarray(b[..., 0]),
            "yc": np.ascontiguousarray(b[..., 1]),
            "zc": np.ascontiguousarray(b[..., 2]),
        })

    res = run_bass_kernel_spmd(nc, in_maps, list(range(N_CORES)))
    out = np.concatenate(
        [res.results[c]["feats"].reshape(C_LOC, 16) for c in range(N_CORES)],
        axis=0)
    return out.astype(np.float32)
